# revision 30
# baseline (speedup 1.0000x reference)
"""CRF forward (log partition) on 8 NeuronCores, data-parallel over batch.

Math: the forward recurrence runs in probability space: with E = exp(T) and
G_t = exp(emissions_t), alpha_{t+1} = logit_t + LSE_j(T + alpha_t) becomes the
linear recurrence P_{t+1} = G_t o (E @ P_t).

All normalization is folded into the DATA on the host: each active step's
emission row is pre-scaled by 1/m_t[b] with m_t[b] = sum_i G[b,t,i]*rowmean(E)_i
(a deterministic per-sequence scalar), which keeps the state O(1) in bf16 range
without any data-dependent renorm on device.  The log-scales are accumulated in
float64 host-side and added back at the end.

Variable lengths via an extra DONE label D per group (46 labels on device):
E'[D,:45] = E[STOP,:], E'[D,D] = 1.0 (exact in bf16), column D otherwise 0.
Active steps emit 0 for D so P[D] stays exactly 0; the absorb step at t=len[b]
emits onehot(D), capturing LSE_j(T[STOP,j]+alpha_j) -- the final answer -- into
P[D]; later steps emit onehot(D) again, multiplying P[D] by exactly 1.0.

TWO staggered chains per core: the per-step serial latency is
~400ns fixed + ~1.8ns/column, so each core's 64 columns are split into two
independent chains (even sorted ranks -> device cols 0..31, odd -> 32..63).
Each chain's link is ~1.8ns/col cheaper at half the width, and the chains
interleave on the PE/DVE queues (both engines have idle slack), cutting wall
time below the single-chain floor.  Widths shrink with a per-chain schedule
derived from the actual length distribution; dead columns freeze in the
in-place per-chain state tiles.

Per-core critical path per chain step: one bf16 matmul [92,92]x[92,n] with the
stationary blockdiag(E'^T,E'^T) kept loaded in the PE array (standalone
ldweights + stripped auto-loads), then one DVE tensor_mul.
"""

import numpy as np
import ml_dtypes

import concourse.bacc as bacc
import concourse.mybir as mybir
import concourse.tile as tile
from concourse.bass_utils import run_bass_kernel_spmd

L = 45
START = 43
STOP = 44
LD = 46                    # labels + DONE landing pad
DONE = 45
B = 1024
S = 512
NCORES = 8
BPC = B // NCORES          # 128 sequences per core
NG = 2                     # groups per core
WCOL = BPC // NG           # 64 columns per group
HALF = WCOL // 2           # 32 columns per chain
PR = NG * LD               # 92 partition rows for packed state
TSTEPS = S + 1             # +1 appended absorb step
NSLOTS = NCORES * NG       # 16 (core, group) slots

F32 = mybir.dt.float32
BF16 = mybir.dt.bfloat16
NP_BF16 = ml_dtypes.bfloat16


class _Plan:
    """Dual-chain width schedules + g layout + chunking."""

    def __init__(self, na, nb):
        na = np.asarray(na, np.int64)
        nb = np.asarray(nb, np.int64)
        assert na.shape == (S,) and nb.shape == (S,)
        assert na[0] == HALF and nb[0] == HALF
        self.na, self.nb = na, nb
        # Lifetime (last updated step) per device column.
        self.t_col = np.zeros(WCOL, np.int64)
        for j in range(HALF):
            self.t_col[j] = int((np.where(na > j)[0] + 1).max()) if (na > j).any() else 0
            self.t_col[HALF + j] = (
                int((np.where(nb > j)[0] + 1).max()) if (nb > j).any() else 0
            )
        # Per-step g block widths (block 0 = full-width init, [A|B] layout).
        self.blk_w = np.concatenate([[WCOL], na + nb])          # [TSTEPS]
        self.blk_off = np.concatenate([[0], np.cumsum(self.blk_w)])
        self.gcols = int(self.blk_off[-1])
        # Chunk boundaries (step indices): small leading chunks start the
        # pipeline fast; later chunks rotate through a 2-buffer pool with
        # lazily issued DMAs so the transfers trail compute.
        self.chunk_steps = [0, 1, 9, 41] + list(
            np.linspace(41, TSTEPS, 9).astype(int)[1:]
        )
        self.nchunk = len(self.chunk_steps) - 1
        self.neager = 3
        self.rot_w = int(
            max(
                self.blk_off[self.chunk_steps[c + 1]] - self.blk_off[self.chunk_steps[c]]
                for c in range(self.neager, self.nchunk)
            )
        )
        self.key = na.tobytes() + nb.tobytes()


def _build_nc(plan):
    # Bacc (not raw Bass): its legalization splits multi-sem waits into
    # standalone event-semaphore instructions, which walrus codegen requires.
    nc = bacc.Bacc("TRN2", target_bir_lowering=False, debug=False, num_devices=NCORES)
    # The stationary e2t matrix rides as the first PR columns of g, so one
    # DMA (and one semaphore) gates both the ldweights and the first matmuls.
    g_dram = nc.dram_tensor("g", [PR, PR + plan.gcols], BF16, kind="ExternalInput")
    wout_dram = nc.dram_tensor("wout", [PR, WCOL], BF16, kind="ExternalOutput")

    cs = plan.chunk_steps
    with tile.TileContext(nc) as tc:
        with (
            tc.tile_pool(name="geager", bufs=1) as ge_pool,
            tc.tile_pool(name="grot", bufs=2) as gr_pool,
            tc.tile_pool(name="state", bufs=1) as state_pool,
            tc.tile_pool(name="ps_s", bufs=3, space="PSUM") as ps_s,
        ):
            gtiles = []
            for c in range(plan.neager):
                c0 = PR + int(plan.blk_off[cs[c]]) if c > 0 else 0
                c1 = PR + int(plan.blk_off[cs[c + 1]])
                gt = ge_pool.tile([PR, c1 - c0], BF16, tag=f"g{c}")
                nc.sync.dma_start(gt[:], g_dram[:, c0:c1])
                gtiles.append(gt)

            e2t = gtiles[0][:, 0:PR]

            # Load blockdiag(E'^T, E'^T) into the PE array once; every step
            # matmul below reuses it (redundant auto-ldweights are stripped
            # after tile legalization below).
            nc.tensor.ldweights(e2t)

            # Per-chain in-place states; step 1 is full width per chain, so
            # both tiles are fully written by the first tensor_muls (the
            # first matmuls read the host-folded W_0 block of g directly).
            w_a = state_pool.tile([PR, HALF], BF16, tag="wa")
            w_b = state_pool.tile([PR, HALF], BF16, tag="wb")

            chunk_of = np.searchsorted(cs, np.arange(TSTEPS), "right") - 1
            next_chunk = plan.neager
            for t in range(1, TSTEPS):
                # Issue each rotating chunk's DMA ~16 steps ahead of use; the
                # 2-buf pool WAR dep keeps transfers trailing compute.
                while next_chunk < plan.nchunk and t >= cs[next_chunk] - 16:
                    cc = next_chunk
                    c0 = PR + int(plan.blk_off[cs[cc]])
                    c1 = PR + int(plan.blk_off[cs[cc + 1]])
                    gt = gr_pool.tile([PR, plan.rot_w], BF16, tag="grot")
                    nc.sync.dma_start(gt[:, 0 : c1 - c0], g_dram[:, c0:c1])
                    gtiles.append(gt)
                    next_chunk += 1
                na = int(plan.na[t - 1])
                nb = int(plan.nb[t - 1])
                c = int(chunk_of[t])
                off = int(plan.blk_off[t] - plan.blk_off[cs[c]])
                if c == 0:
                    off += PR
                gt = gtiles[c]
                mova = gtiles[0][:, PR : PR + HALF] if t == 1 else w_a[:, 0:na]
                movb = (
                    gtiles[0][:, PR + HALF : PR + WCOL] if t == 1 else w_b[:, 0:nb]
                )
                ps_a = ps_s.tile([PR, HALF], F32, tag="sa")
                nc.tensor.matmul(ps_a[:, 0:na], e2t, mova, start=True, stop=True)
                if nb > 0:
                    ps_b = ps_s.tile([PR, HALF], F32, tag="sb")
                    nc.tensor.matmul(ps_b[:, 0:nb], e2t, movb, start=True, stop=True)
                nc.vector.tensor_mul(
                    w_a[:, 0:na], gt[:, off : off + na], ps_a[:, 0:na]
                )
                if nb > 0:
                    nc.vector.tensor_mul(
                        w_b[:, 0:nb], gt[:, off + na : off + na + nb], ps_b[:, 0:nb]
                    )

            nc.sync.dma_start(wout_dram[:, 0:HALF], w_a[:])
            nc.sync.dma_start(wout_dram[:, HALF:WCOL], w_b[:])

    # Tile legalization splits every bf16 matmult into LDWEIGHTS + MATMULT.
    # All those loads are of the SAME stationary tile, so keep only the
    # first (the explicit one above) and drop the rest.  The auto-inserted
    # loads carry no semaphore waits/updates (all sync lives on the
    # matmults), so removal is sync-neutral.
    kept_first = False
    for blk in nc.main_func.blocks:
        for i in list(blk.instructions):
            if isinstance(i, mybir.InstLdweights):
                if not kept_first:
                    kept_first = True
                elif i.sync_info is None:
                    blk.instructions.remove(i)

    nc.compile()
    return nc


_NC_CACHE = {}


def _get_nc():
    """Return the nc built for the most recent _prep_inputs call."""
    return _NC_CACHE["nc"]


def _host_norm(logit_b, len_b, T):
    """Exact float64 log-space forward for one sequence (fallback path)."""
    NEG_INF = -10000.0
    alpha = np.full(L, NEG_INF)
    alpha[START] = 0.0
    for t in range(len_b):
        mat = T + alpha[None, :]
        mx = mat.max(axis=1)
        alpha = logit_b[t] + np.log(np.exp(mat - mx[:, None]).sum(axis=1)) + mx
    v = alpha + T[STOP]
    mx = v.max()
    return np.log(np.exp(v - mx).sum()) + mx


def _prep_inputs(logits, lens, transitions):
    """Host-side preprocessing: exp + absorb-rewrite + deterministic
    per-(seq,step) scaling + dual-chain length-sorted packing."""
    logits = np.asarray(logits, np.float32)
    lens = np.asarray(lens, np.int64)
    T = np.asarray(transitions, np.float64)

    E = np.exp(T)                      # [45,45] float64
    erow = E.mean(axis=1)              # mean_j E[i,j], [45]

    Eg = np.zeros((LD, LD), np.float64)
    Eg[:L, :L] = E
    Eg[DONE, :L] = E[STOP, :]
    Eg[DONE, DONE] = 1.0
    e2t = np.zeros((PR, PR), np.float64)
    e2t[:LD, :LD] = Eg.T
    e2t[LD:, LD:] = Eg.T

    G = np.exp(logits.astype(np.float64))          # [B,S,45]

    t_idx = np.arange(S)[None, :]                  # [1,S]
    active = t_idx < lens[:, None]                 # [B,S]

    # Fold step 0 and normalize it exactly: W0 = G0*E[:,START], scale 1/sum.
    W0 = G[:, 0, :] * E[:, START][None, :]         # [B,45]
    m0 = W0.sum(axis=1)                            # [B]
    G[:, 0, :] = W0 / m0[:, None]

    # Active steps t>=1: scale by 1/m_t, m_t = sum_i G_t[i]*erow[i].
    m = G @ erow                                   # [B,S]
    scale_mask = active & (t_idx > 0)
    np.divide(G, m[:, :, None], out=G, where=scale_mask[:, :, None])

    # log-scale accumulator: z[b] = log m0 + sum_{1<=t<len} log m_t.
    logm = np.where(scale_mask, np.log(m), 0.0)
    z = np.log(m0) + logm.sum(axis=1)

    # 46-label emissions: D gets 0 while active, onehot(D) from t>=len on.
    G46 = np.zeros((B, TSTEPS, LD), np.float64)
    G46[:, :S, :L] = np.where(active[:, :, None], G, 0.0)
    done_from = t_idx >= lens[:, None]             # includes absorb step
    G46[:, :S, DONE] = np.where(done_from, 1.0, 0.0)
    G46[:, S, DONE] = 1.0                          # appended step

    # Deal longest-first round-robin across the 16 (core, group) slots, then
    # split each slot's sorted ranks into chain A (even) / chain B (odd):
    # device cols 0..31 hold ranks 0,2,..,62 and cols 32..63 ranks 1,3,..,63.
    order = np.argsort(-lens, kind="stable")
    slots = np.empty((NSLOTS, WCOL), np.int64)
    for r, b in enumerate(order):
        slots[r % NSLOTS][r // NSLOTS] = b
    perm = np.concatenate([np.arange(0, WCOL, 2), np.arange(1, WCOL, 2)])
    slots_dev = slots[:, perm]                     # [NSLOTS, WCOL] device order
    lens_dev = lens[slots_dev]

    # Exact per-chain width schedules (max over slots), step 1 full width.
    steps = np.arange(1, TSTEPS)                   # [S]
    na = (lens_dev[:, :HALF, None] >= steps[None, None, :]).sum(axis=1).max(axis=0)
    nb = (lens_dev[:, HALF:, None] >= steps[None, None, :]).sum(axis=1).max(axis=0)
    na = np.maximum(na, 1)
    na[0] = HALF
    nb[0] = HALF
    na = np.maximum.accumulate(na[::-1])[::-1]
    nb = np.maximum.accumulate(nb[::-1])[::-1]
    plan = _Plan(na, nb)

    if _NC_CACHE.get("key") != plan.key:
        _NC_CACHE["nc"] = _build_nc(plan)
        _NC_CACHE["key"] = plan.key

    # Host fallback for any sequence outliving its device column (none when
    # the schedule is derived from these lens, but guards arbitrary inputs).
    host_norms = {}
    logits64 = logits.astype(np.float64)
    for s in range(NSLOTS):
        for k in range(WCOL):
            b = slots_dev[s][k]
            if lens[b] > plan.t_col[k]:
                host_norms[int(b)] = _host_norm(logits64[b], int(lens[b]), T)

    _NC_CACHE["plan"] = plan
    _NC_CACHE["z"] = z
    _NC_CACHE["slots_dev"] = slots_dev
    _NC_CACHE["host_norms"] = host_norms

    g16 = G46.astype(NP_BF16)
    e2t16 = e2t.astype(NP_BF16)
    in_maps = []
    for c in range(NCORES):
        g_in = np.zeros((PR, PR + plan.gcols), NP_BF16)
        g_in[:, :PR] = e2t16
        for g in range(NG):
            seqs = slots_dev[c * NG + g]           # [WCOL] device col -> seq
            rows = slice(g * LD, (g + 1) * LD)
            gc = g16[seqs]                         # [WCOL, TSTEPS, LD]
            # Init block: [A cols 0..31 | B cols 32..63], full width.
            g_in[rows, PR : PR + WCOL] = gc[:, 0, :].T
            for t in range(1, TSTEPS):
                wa = int(plan.na[t - 1])
                wb = int(plan.nb[t - 1])
                o = PR + int(plan.blk_off[t])
                g_in[rows, o : o + wa] = gc[:wa, t, :].T
                g_in[rows, o + wa : o + wa + wb] = gc[HALF : HALF + wb, t, :].T
        in_maps.append({"g": g_in})
    return in_maps


def _postprocess(results, lens, transitions):
    z = _NC_CACHE["z"]
    slots_dev = _NC_CACHE["slots_dev"]
    host_norms = _NC_CACHE["host_norms"]
    norm = np.empty(B, np.float64)
    for c in range(NCORES):
        wout = np.asarray(results[c]["wout"]).astype(np.float64)  # [PR, WCOL]
        for g in range(NG):
            seqs = slots_dev[c * NG + g]
            pdone = wout[g * LD + DONE, :]
            norm[seqs] = np.log(pdone) + z[seqs]
    for b, v in host_norms.items():
        norm[b] = v
    return norm.astype(np.float32)


def kernel(logits, lens, transitions):
    in_maps = _prep_inputs(logits, lens, transitions)
    nc = _get_nc()
    res = run_bass_kernel_spmd(nc, in_maps, list(range(NCORES)))
    return _postprocess(res.results, lens, transitions)
